# revision 11
# baseline (speedup 1.0000x reference)
"""Trainium2 Bass kernel for ConvPolicy14 (dense_cnn, 93 -> 40 policy net).

Strategy: the network is tiny (~4.6K MACs), so every conv/pool/upsample/concat
is folded (input-independently, on host) into a chain of 8 small dense affine
layers executed as TensorE matmuls with PSUM accumulation:

    h1 = tanh(M1 v0 + b1)            v0 = jcat flattened (84)
    h2 = tanh(M2 h1 + b2)
    h3 = tanh(M3 h2 + b3)            M3 = conv3_toeplitz @ avgpool
    h4 = tanh(M4 h3 + b4)
    h5 = tanh(M5 (h4 + ext) + b5)    ext = [psi, x47, x52]; split into 2 matmuls
    h6 = tanh(M6a h5 + M6b h3 + b6)  concat -> accumulating matmul pair
    h7 = tanh(M7a h6 + M7b h2 + b7)  M7a folds nearest-upsample
    out = M8a h7 + M8b v0 + b8       (40,) final, no tanh

Biases are folded into the matmul accumulation groups as extra contraction
rows against constant-1.0 cells (v0 has structural zeros reused as the 1.0
slot for layers 1/8), so ScalarE does pure tanh with no operand loads and
every instruction carries at most one sync wait (walrus S3_LW limit).
psi = atan2(qz,qw) - atan2(-qx,qy) is computed on-device (DVE reciprocal /
compares + one ACT Arctan, branchless quadrant fix) off the critical path.
All ACT functions used (Tanh/Arctan/Copy) live in one table set so a single
~2.7us ACT table load is paid. One input DMA, one output DMA.

Per the sharding hint the problem is too small to shard: all 8 cores run the
same program; core 0's output is returned.
"""

import numpy as np

F32 = np.float32

# ---------------------------------------------------------------------------
# Block layout (single DMA'd constant/input block, 128 partitions x _B_COLS)
# ---------------------------------------------------------------------------
_C_L1 = 0        # lhsT1 (84, 28); row0 = b1 (v0[0] := 1.0 trick)
_C_L2 = 28       # lhsT2 (28, 28)
_C_B2 = 56       # b2 row (1, 28)
_C_L3 = 84       # lhsT3 (28, 12)
_C_B3 = 96       # b3 row (1, 12)
_C_L4 = 108      # lhsT4 (12, 3)
_C_B4 = 111      # b4 row (1, 3)
_C_L5A = 114     # lhsT5a (3, 12)
_C_L5B = 126     # lhsT5b (4, 12); row3 = b5
_C_L6A = 138     # lhsT6a (12, 12)
_C_L6B = 150     # lhsT6b (12, 12)
_C_B6 = 162      # b6 row (1, 12)
_C_L7A = 174     # lhsT7a (12, 28)
_C_L7B = 202     # lhsT7b (28, 28)
_C_B7 = 230      # b7 row (1, 28)
_C_L8A = 258     # lhsT8a (28, 40)
_C_L8B = 298     # lhsT8b (84, 40); row0 = b8
_XO = 338        # x block start
_C_V0 = _XO + 0      # v0 col (p0 = 1.0, p2-41 = x[7:47], p44-83 = x[53:93])
_C_EXT = _XO + 1     # ext col (p0 = psi slot, p1 = x47, p2 = x52, p3 = 1.0)
_C_DEN = _XO + 2     # [qw, qy] at p0, 2 cols
_C_NUM = _XO + 4     # [qz, qx] at p0, 2 cols
_C_ONE = _XO + 6     # 1.0 at p0
_B_COLS = _XO + 8


def _toeplitz_conv(cw, L):
    """Conv1d pad=1 k=3: out[(o,l)] = sum_{c,k} cw[o,c,k] x[(c,l+k-1)]."""
    O, C, _ = cw.shape
    M = np.zeros((O * L, C * L), F32)
    for o in range(O):
        for l in range(L):
            for c in range(C):
                for k in range(3):
                    m = l + k - 1
                    if 0 <= m < L:
                        M[o * L + l, c * L + m] = cw[o, c, k]
    return M


def _toeplitz_deconv(dw, L):
    """ConvTranspose1d pad=1 k=3 s=1: out[(o,l)] = sum dw[c,o,1-m+l] x[(c,m)]."""
    C, O, _ = dw.shape
    M = np.zeros((O * L, C * L), F32)
    for o in range(O):
        for l in range(L):
            for c in range(C):
                for m in range(L):
                    k = 1 - m + l
                    if 0 <= k < 3:
                        M[o * L + l, c * L + m] = dw[c, o, k]
    return M


def _build_w_block(w):
    c1w, c1b = w["conv1_w"], w["conv1_b"]
    c2w, c2b = w["conv2_w"], w["conv2_b"]
    c3w, c3b = w["conv3_w"], w["conv3_b"]
    c4w, c4b = w["conv4_w"], w["conv4_b"]
    d1w, d1b = w["deconv1_w"], w["deconv1_b"]
    d2w, d2b = w["deconv2_w"], w["deconv2_b"]
    d3w, d3b = w["deconv3_w"], w["deconv3_b"]
    d4w, d4b = w["deconv4_w"], w["deconv4_b"]

    M1 = _toeplitz_conv(c1w, 7)                     # (28, 84)
    M2 = _toeplitz_conv(c2w, 7)                     # (28, 28)

    # adaptive avg pool (4,7)->(4,3), windows [0:3],[2:5],[4:7]
    P = np.zeros((12, 28), F32)
    for c in range(4):
        for j in range(3):
            P[c * 3 + j, c * 7 + 2 * j: c * 7 + 2 * j + 3] = 1.0 / 3.0
    T3 = np.zeros((12, 12), F32)
    for o in range(4):
        for j in range(3):
            for c in range(4):
                for k in range(3):
                    jp = j + k - 1
                    if 0 <= jp < 3:
                        T3[o * 3 + j, c * 3 + jp] = c3w[o, c, k]
    M3 = (T3.astype(np.float64) @ P.astype(np.float64)).astype(F32)  # (12, 28)

    M4 = np.zeros((3, 12), F32)                     # conv4 pad0 L3->1
    for o in range(3):
        for c in range(4):
            M4[o, c * 3: c * 3 + 3] = c4w[o, c, :]

    M5 = np.zeros((12, 3), F32)                     # deconv1 L1->3
    for o in range(4):
        for l in range(3):
            for c in range(3):
                M5[o * 3 + l, c] = d1w[c, o, l]

    M6 = _toeplitz_deconv(d2w, 3)                   # (12, 24)

    T7 = _toeplitz_deconv(d3w, 7)                   # (28, 56)
    g = [0, 0, 0, 1, 1, 2, 2]                       # nearest-upsample 3->7
    U = np.zeros((28, 12), F32)
    for c in range(4):
        for l in range(7):
            U[c * 7 + l, c * 3 + g[l]] = 1.0
    M7a = (T7[:, :28].astype(np.float64) @ U.astype(np.float64)).astype(F32)
    M7b = np.ascontiguousarray(T7[:, 28:])          # (28, 28)

    M8 = _toeplitz_deconv(d4w, 7)[2:, :]            # (40, 112): acts[2:]
    b8 = np.repeat(d4b, 7).astype(F32)[2:]

    b1 = np.repeat(c1b, 7).astype(F32)
    b2 = np.repeat(c2b, 7).astype(F32)
    b3 = np.repeat(c3b, 3).astype(F32)
    b5 = np.repeat(d1b, 3).astype(F32)
    b6 = np.repeat(d2b, 3).astype(F32)
    b7 = np.repeat(d3b, 7).astype(F32)

    # v0[0] / v0[1] are structural zeros; v0[0] is repurposed as a 1.0 cell.
    # Zero the (irrelevant) weight columns and plant biases there.
    lhsT1 = M1.T.copy()                 # (84, 28)
    lhsT1[0, :] = b1
    lhsT1[1, :] = 0.0
    M8b = M8[:, 28:].copy()             # (40, 84) weights on v0
    lhsT8b = M8b.T.copy()               # (84, 40)
    lhsT8b[0, :] = b8
    lhsT8b[1, :] = 0.0
    lhsT5b = np.concatenate([M5.T, b5[None, :]], axis=0)  # (4, 12)

    blk = np.zeros((128, _B_COLS), F32)

    def put(col, mat):
        K, M = mat.shape
        blk[:K, col:col + M] = mat

    put(_C_L1, lhsT1)
    put(_C_L2, M2.T)
    put(_C_B2, b2[None, :])
    put(_C_L3, M3.T)
    put(_C_B3, b3[None, :])
    put(_C_L4, M4.T)
    put(_C_B4, np.asarray(c4b, F32)[None, :])
    put(_C_L5A, M5.T)
    put(_C_L5B, lhsT5b)
    put(_C_L6A, M6[:, :12].T)
    put(_C_L6B, M6[:, 12:].T)
    put(_C_B6, b6[None, :])
    put(_C_L7A, M7a.T)
    put(_C_L7B, M7b.T)
    put(_C_B7, b7[None, :])
    put(_C_L8A, M8[:, :28].T)
    put(_C_L8B, lhsT8b)
    return blk


def _fill_x_block(blk, x):
    x = np.asarray(x, F32).reshape(-1)
    blk[:, _XO:] = 0.0
    blk[0, _C_V0] = 1.0                 # constant-1 slot (v0[0] structural 0)
    blk[2:42, _C_V0] = x[7:47]
    blk[44:84, _C_V0] = x[53:93]
    blk[1, _C_EXT] = x[47]              # p0 left 0: psi computed on device
    blk[2, _C_EXT] = x[52]
    blk[3, _C_EXT] = 1.0                # bias slot for layer 5
    blk[0, _C_DEN] = x[3]               # qw
    blk[0, _C_DEN + 1] = x[5]           # qy
    blk[0, _C_NUM] = x[6]               # qz
    blk[0, _C_NUM + 1] = x[4]           # qx
    blk[0, _C_ONE] = 1.0
    return blk


_CACHE = {}


def _build_bass():
    if "nc" in _CACHE:
        return _CACHE["nc"]

    import concourse.mybir as mybir
    from concourse import bacc, tile

    f32 = mybir.dt.float32
    AF = mybir.ActivationFunctionType
    OP = mybir.AluOpType

    nc = bacc.Bacc("TRN2", num_devices=8)
    b_dram = nc.declare_dram_parameter("blk", [128, _B_COLS], f32, isOutput=False)
    out_dram = nc.declare_dram_parameter("out", [40, 1], f32, isOutput=True)

    with tile.TileContext(nc) as tc:
        with (
            tc.tile_pool(name="sbuf", bufs=1) as pool,
            tc.tile_pool(name="psum", bufs=1, space="PSUM") as psum,
        ):
            Bt = pool.tile([128, _B_COLS], f32, name="Bt", tag="bt")
            Ht = pool.tile([128, 9], f32, name="Ht", tag="ht")
            St = pool.tile([128, 24], f32, name="St", tag="st")

            nc.sync.dma_start(Bt[:, :], b_dram[:, :])

            dims = [28, 28, 12, 3, 12, 12, 28, 40]
            ps = [psum.tile([m, 1], f32, name=f"ps{i}", tag=f"ps{i}")
                  for i, m in enumerate(dims)]

            def mm(i, lhs_col, k, m, rhs, start, stop):
                nc.tensor.matmul(ps[i][:, :], Bt[0:k, lhs_col:lhs_col + m],
                                 rhs, start=start, stop=stop)

            def tanh_to(i, m, hcol, func=AF.Tanh):
                nc.scalar.activation(Ht[0:m, hcol:hcol + 1], ps[i][0:m, 0:1],
                                     func, bias=0.0, scale=1.0)

            one = Bt[0:1, _C_ONE:_C_ONE + 1]
            EXTC = 7   # Ht col: ext vector [psi, x47, x52, 1.0]
            OUTC = 8   # Ht col: final output

            # --- atan2 DVE stage (ready right after the DMA). q is emitted
            # LAST so arctan's single DVE wait covers the whole chain. ---
            nc.vector.reciprocal(St[0:1, 0:2], Bt[0:1, _C_DEN:_C_DEN + 2])
            nc.vector.tensor_scalar(St[0:1, 4:6], Bt[0:1, _C_DEN:_C_DEN + 2],
                                    0.0, None, OP.is_lt)            # [den<0]
            nc.vector.tensor_scalar(St[0:1, 6:8], Bt[0:1, _C_NUM:_C_NUM + 2],
                                    0.0, None, OP.is_ge)            # [num>=0]
            nc.vector.tensor_scalar(St[0:1, 8:10], St[0:1, 6:8],
                                    2.0, -1.0, OP.mult, OP.add)     # sign(num)
            nc.vector.tensor_mul(St[0:1, 10:12], St[0:1, 8:10], St[0:1, 4:6])
            nc.vector.tensor_scalar(St[0:1, 12:14], St[0:1, 10:12],
                                    float(np.pi / 2), None, OP.mult)
            nc.vector.tensor_add(St[0:1, 14:15], St[0:1, 12:13],
                                 St[0:1, 13:14])                    # b = corr/2
            nc.vector.tensor_mul(St[0:1, 2:4], Bt[0:1, _C_NUM:_C_NUM + 2],
                                 St[0:1, 0:2])                      # q = num/den

            # --- L1 (bias in lhsT1 row 0 against v0[0]=1.0) ---
            mm(0, _C_L1, 84, 28, Bt[0:84, _C_V0:_C_V0 + 1], True, True)
            tanh_to(0, 28, 0)
            # ext tail [x47, x52, 1.0] -> Ht; also gives ACT its one DMA wait
            nc.scalar.activation(Ht[0:4, EXTC:EXTC + 1],
                                 Bt[0:4, _C_EXT:_C_EXT + 1], AF.Copy,
                                 bias=0.0, scale=1.0)
            # --- L2 ---
            mm(1, _C_B2, 1, 28, one, True, False)
            mm(1, _C_L2, 28, 28, Ht[0:28, 0:1], False, True)
            tanh_to(1, 28, 1)

            # --- atan2 ACT tail in the tanh gaps:
            # psi = sum_f(arctan(q_f) + b) via Identity + accum_out.
            # b is bounced through an ACT copy so the psi op's waits stay
            # single-proc (walrus allows one sync wait per instruction). ---
            nc.scalar.activation(St[0:1, 20:21], St[0:1, 14:15], AF.Copy,
                                 bias=0.0, scale=1.0)
            nc.scalar.activation(St[0:1, 16:18], St[0:1, 2:4], AF.Arctan,
                                 bias=0.0, scale=1.0)
            nc.scalar.activation(St[0:1, 18:20], St[0:1, 16:18], AF.Identity,
                                 bias=St[0:1, 20:21], scale=1.0,
                                 accum_out=Ht[0:1, EXTC:EXTC + 1])

            # --- L3 ---
            mm(2, _C_B3, 1, 12, one, True, False)
            mm(2, _C_L3, 28, 12, Ht[0:28, 1:2], False, True)
            tanh_to(2, 12, 2)
            # --- L4 ---
            mm(3, _C_B4, 1, 3, one, True, False)
            mm(3, _C_L4, 12, 3, Ht[0:12, 2:3], False, True)
            tanh_to(3, 3, 3)
            # --- L5: M5 (h4 + ext) + b5 ---
            mm(4, _C_L5B, 4, 12, Ht[0:4, EXTC:EXTC + 1], True, False)
            mm(4, _C_L5A, 3, 12, Ht[0:3, 3:4], False, True)
            tanh_to(4, 12, 4)
            # --- L6 ---
            mm(5, _C_B6, 1, 12, one, True, False)
            mm(5, _C_L6B, 12, 12, Ht[0:12, 2:3], False, False)
            mm(5, _C_L6A, 12, 12, Ht[0:12, 4:5], False, True)
            tanh_to(5, 12, 5)
            # --- L7 ---
            mm(6, _C_B7, 1, 28, one, True, False)
            mm(6, _C_L7B, 28, 28, Ht[0:28, 1:2], False, False)
            mm(6, _C_L7A, 12, 28, Ht[0:12, 5:6], False, True)
            tanh_to(6, 28, 6)
            # --- L8 (bias in lhsT8b row 0; no tanh) ---
            mm(7, _C_L8B, 84, 40, Bt[0:84, _C_V0:_C_V0 + 1], True, False)
            mm(7, _C_L8A, 28, 40, Ht[0:28, 6:7], False, True)
            tanh_to(7, 40, OUTC, func=AF.Copy)

            nc.sync.dma_start(out_dram[:, :], Ht[0:40, OUTC:OUTC + 1])

    nc.compile()
    _CACHE["nc"] = nc
    return nc


def _build_blk(inputs):
    blk = _build_w_block(inputs)
    _fill_x_block(blk, inputs["x"])
    return blk


def kernel(**inputs) -> np.ndarray:
    nc = _build_bass()
    blk = _build_blk(inputs)

    from concourse.bass_utils import run_bass_kernel_spmd

    res = run_bass_kernel_spmd(nc, [{"blk": blk.copy()} for _ in range(8)],
                               core_ids=list(range(8)))
    out = np.asarray(res.results[0]["out"], F32).reshape(1, 40)
    return out


# revision 14
# speedup vs baseline: 1.0968x; 1.0968x over previous
"""Trainium2 Bass kernel for ConvPolicy14 (dense_cnn, 93 -> 40 policy net).

Strategy: the network is tiny (~4.6K MACs), so every conv/pool/upsample/concat
is folded (input-independently, on host) into a chain of 8 small dense affine
layers executed as TensorE matmuls with PSUM accumulation:

    h1 = tanh(M1 v0 + b1)            v0 = jcat flattened (84)
    h2 = tanh(M2 h1 + b2)
    h3 = tanh(M3 h2 + b3)            M3 = conv3_toeplitz @ avgpool
    h4 = tanh(M4 h3 + b4)
    h5 = tanh(M5 (h4 + ext) + b5)    ext = [psi, x47, x52]; split into 2 matmuls
    h6 = tanh(M6a h5 + M6b h3 + b6)  concat -> accumulating matmul pair
    h7 = tanh(M7a h6 + M7b h2 + b7)  M7a folds nearest-upsample
    out = M8a h7 + M8b v0 + b8       (40,) final, no tanh

Biases are folded into the matmul accumulation groups as extra contraction
rows against constant-1.0 cells (v0 has structural zeros reused as the 1.0
slot for layers 1/8), so ScalarE does pure tanh with no operand loads and
every instruction carries at most one sync wait (walrus S3_LW limit).
psi = atan2(qz,qw) - atan2(-qx,qy) is computed on-device (DVE reciprocal /
compares + one ACT Arctan, branchless quadrant fix) off the critical path.
All ACT functions used (Tanh/Arctan/Copy) live in one table set so a single
~2.7us ACT table load is paid. One input DMA, one output DMA.

Per the sharding hint the problem is too small to shard: all 8 cores run the
same program; core 0's output is returned.
"""

import numpy as np

F32 = np.float32

# ---------------------------------------------------------------------------
# Block layout (single DMA'd constant/input block, 128 partitions x _B_COLS)
# ---------------------------------------------------------------------------
_C_L1 = 0        # lhsT1 (84, 28); row0 = b1 (v0[0] := 1.0 trick)
_C_L2 = 28       # lhsT2 (28, 28)
_C_B2 = 56       # b2 row (1, 28)
_C_L3 = 84       # lhsT3 (28, 12)
_C_B3 = 96       # b3 row (1, 12)
_C_L4 = 108      # lhsT4 (12, 3)
_C_B4 = 111      # b4 row (1, 3)
_C_L5A = 114     # lhsT5a (3, 12)
_C_L5B = 126     # lhsT5b (4, 12); row3 = b5
_C_L6A = 138     # lhsT6a (12, 12)
_C_L6B = 150     # lhsT6b (12, 12)
_C_B6 = 162      # b6 row (1, 12)
_C_L7A = 174     # lhsT7a (12, 28)
_C_L7B = 202     # lhsT7b (28, 28)
_C_B7 = 230      # b7 row (1, 28)
_C_L8A = 258     # lhsT8a (28, 40)
_C_L8B = 298     # lhsT8b (84, 40); row0 = b8
_XO = 338        # x block start
_C_V0 = _XO + 0      # v0 col (p0 = 1.0, p2-41 = x[7:47], p44-83 = x[53:93])
_C_EXT = _XO + 1     # ext col (p0 = psi slot, p1 = x47, p2 = x52, p3 = 1.0)
_C_DEN = _XO + 2     # [qw, qy] at p0, 2 cols
_C_NUM = _XO + 4     # [qz, qx] at p0, 2 cols
_C_ONE = _XO + 6     # 1.0 at p0
_B_COLS = _XO + 8


def _toeplitz_conv(cw, L):
    """Conv1d pad=1 k=3: out[(o,l)] = sum_{c,k} cw[o,c,k] x[(c,l+k-1)]."""
    O, C, _ = cw.shape
    M = np.zeros((O * L, C * L), F32)
    for o in range(O):
        for l in range(L):
            for c in range(C):
                for k in range(3):
                    m = l + k - 1
                    if 0 <= m < L:
                        M[o * L + l, c * L + m] = cw[o, c, k]
    return M


def _toeplitz_deconv(dw, L):
    """ConvTranspose1d pad=1 k=3 s=1: out[(o,l)] = sum dw[c,o,1-m+l] x[(c,m)]."""
    C, O, _ = dw.shape
    M = np.zeros((O * L, C * L), F32)
    for o in range(O):
        for l in range(L):
            for c in range(C):
                for m in range(L):
                    k = 1 - m + l
                    if 0 <= k < 3:
                        M[o * L + l, c * L + m] = dw[c, o, k]
    return M


def _build_w_block(w):
    c1w, c1b = w["conv1_w"], w["conv1_b"]
    c2w, c2b = w["conv2_w"], w["conv2_b"]
    c3w, c3b = w["conv3_w"], w["conv3_b"]
    c4w, c4b = w["conv4_w"], w["conv4_b"]
    d1w, d1b = w["deconv1_w"], w["deconv1_b"]
    d2w, d2b = w["deconv2_w"], w["deconv2_b"]
    d3w, d3b = w["deconv3_w"], w["deconv3_b"]
    d4w, d4b = w["deconv4_w"], w["deconv4_b"]

    M1 = _toeplitz_conv(c1w, 7)                     # (28, 84)
    M2 = _toeplitz_conv(c2w, 7)                     # (28, 28)

    # adaptive avg pool (4,7)->(4,3), windows [0:3],[2:5],[4:7]
    P = np.zeros((12, 28), F32)
    for c in range(4):
        for j in range(3):
            P[c * 3 + j, c * 7 + 2 * j: c * 7 + 2 * j + 3] = 1.0 / 3.0
    T3 = np.zeros((12, 12), F32)
    for o in range(4):
        for j in range(3):
            for c in range(4):
                for k in range(3):
                    jp = j + k - 1
                    if 0 <= jp < 3:
                        T3[o * 3 + j, c * 3 + jp] = c3w[o, c, k]
    M3 = (T3.astype(np.float64) @ P.astype(np.float64)).astype(F32)  # (12, 28)

    M4 = np.zeros((3, 12), F32)                     # conv4 pad0 L3->1
    for o in range(3):
        for c in range(4):
            M4[o, c * 3: c * 3 + 3] = c4w[o, c, :]

    M5 = np.zeros((12, 3), F32)                     # deconv1 L1->3
    for o in range(4):
        for l in range(3):
            for c in range(3):
                M5[o * 3 + l, c] = d1w[c, o, l]

    M6 = _toeplitz_deconv(d2w, 3)                   # (12, 24)

    T7 = _toeplitz_deconv(d3w, 7)                   # (28, 56)
    g = [0, 0, 0, 1, 1, 2, 2]                       # nearest-upsample 3->7
    U = np.zeros((28, 12), F32)
    for c in range(4):
        for l in range(7):
            U[c * 7 + l, c * 3 + g[l]] = 1.0
    M7a = (T7[:, :28].astype(np.float64) @ U.astype(np.float64)).astype(F32)
    M7b = np.ascontiguousarray(T7[:, 28:])          # (28, 28)

    M8 = _toeplitz_deconv(d4w, 7)[2:, :]            # (40, 112): acts[2:]
    b8 = np.repeat(d4b, 7).astype(F32)[2:]

    b1 = np.repeat(c1b, 7).astype(F32)
    b2 = np.repeat(c2b, 7).astype(F32)
    b3 = np.repeat(c3b, 3).astype(F32)
    b5 = np.repeat(d1b, 3).astype(F32)
    b6 = np.repeat(d2b, 3).astype(F32)
    b7 = np.repeat(d3b, 7).astype(F32)

    # v0[0] / v0[1] are structural zeros; v0[0] is repurposed as a 1.0 cell.
    # Zero the (irrelevant) weight columns and plant biases there.
    lhsT1 = M1.T.copy()                 # (84, 28)
    lhsT1[0, :] = b1
    lhsT1[1, :] = 0.0
    M8b = M8[:, 28:].copy()             # (40, 84) weights on v0
    lhsT8b = M8b.T.copy()               # (84, 40)
    lhsT8b[0, :] = b8
    lhsT8b[1, :] = 0.0
    lhsT5b = np.concatenate([M5.T, b5[None, :]], axis=0)  # (4, 12)

    blk = np.zeros((128, _B_COLS), F32)

    def put(col, mat):
        K, M = mat.shape
        blk[:K, col:col + M] = mat

    put(_C_L1, lhsT1)
    put(_C_L2, M2.T)
    put(_C_B2, b2[None, :])
    put(_C_L3, M3.T)
    put(_C_B3, b3[None, :])
    put(_C_L4, M4.T)
    put(_C_B4, np.asarray(c4b, F32)[None, :])
    put(_C_L5A, M5.T)
    put(_C_L5B, lhsT5b)
    put(_C_L6A, M6[:, :12].T)
    put(_C_L6B, M6[:, 12:].T)
    put(_C_B6, b6[None, :])
    put(_C_L7A, M7a.T)
    put(_C_L7B, M7b.T)
    put(_C_B7, b7[None, :])
    put(_C_L8A, M8[:, :28].T)
    put(_C_L8B, lhsT8b)
    return blk


def _fill_x_block(blk, x):
    x = np.asarray(x, F32).reshape(-1)
    blk[:, _XO:] = 0.0
    blk[0, _C_V0] = 1.0                 # constant-1 slot (v0[0] structural 0)
    blk[2:42, _C_V0] = x[7:47]
    blk[44:84, _C_V0] = x[53:93]
    blk[1, _C_EXT] = x[47]              # p0 left 0: psi computed on device
    blk[2, _C_EXT] = x[52]
    blk[3, _C_EXT] = 1.0                # bias slot for layer 5
    blk[0, _C_DEN] = x[3]               # qw
    blk[0, _C_DEN + 1] = x[5]           # qy
    blk[0, _C_NUM] = x[6]               # qz
    blk[0, _C_NUM + 1] = x[4]           # qx
    blk[0, _C_ONE] = 1.0
    return blk


_CACHE = {}


def _build_bass():
    if "nc" in _CACHE:
        return _CACHE["nc"]

    import concourse.mybir as mybir
    from concourse import bacc, tile

    f32 = mybir.dt.float32
    AF = mybir.ActivationFunctionType
    OP = mybir.AluOpType

    class _OneSetBacc(bacc.Bacc):
        """Force every activation to resolve to sigmoid_and_others (it covers
        Tanh/Arctan/Copy/Identity) so only one ~2.7us ACT table load is paid.
        Canonical set order (= act_func_set_id) is preserved; other sets just
        stop advertising the functions we use."""

        def insert_act_table_loads(self):
            import bass_rust as _bass_rust
            from concourse.hw_specs import get_activation_tables

            has_activation = any(
                isinstance(i, mybir.InstActivation)
                for b in self.main_func.blocks
                for i in b.instructions
            )
            if not has_activation:
                return
            tables = list(get_activation_tables(self.m.arch).items())
            ours = dict(tables)["sigmoid_and_others"]
            for f in (AF.Tanh, AF.Arctan, AF.Copy, AF.Identity):
                assert f in ours, f
            tables = [(n, (fns if n == "sigmoid_and_others" else fns - ours))
                      for n, fns in tables]
            _bass_rust.insert_act_table_loads(self, tables)

    class _SlimTile(tile.TileContext):
        """Replace Tile's kernel tail (2 all-engine barriers + DMA-ring reset,
        ~5us on HW) with: gated drain -> gpsimd range sem_clear. The drain
        already waits on every proc's final tick, so clearing is safe once it
        completes; sems still end at 0 for re-execution."""

        def _drain_and_barrier(self, tick_clock, wait_clock):
            from concourse.vector_clock import ScopedClock
            from concourse.bass import compact_to_ranges

            nc = self.nc
            drain_inst = nc.sync.drain()
            wait_clock.add_sem_waits(
                drain_inst.ins, ScopedClock({None: tick_clock.global_clock})
            )
            done = nc.alloc_semaphore(f"slim_done_{nc.next_id()}")
            drain_inst.then_inc(done)
            popped = nc._tile_sem_poison_stack.pop()
            assert popped is self._sem_poison
            nc.gpsimd.wait_ge(done, 1)
            sems = list(self.sems.allocated().values())
            sem_nums = [s.num if hasattr(s, "num") else int(s) for s in sems]
            sem_nums.append(done.num)
            for r in compact_to_ranges(sorted(sem_nums)):
                nc.gpsimd.sem_clear(r)
            nc._state.prepend_free_semaphores(sem_nums)
            for ps in nc._tile_sem_poison_stack:
                ps.update(sem_nums)

    nc = _OneSetBacc("TRN2", num_devices=8)
    b_dram = nc.declare_dram_parameter("blk", [128, _B_COLS], f32, isOutput=False)
    out_dram = nc.declare_dram_parameter("out", [40, 1], f32, isOutput=True)

    with _SlimTile(nc) as tc:
        with (
            tc.tile_pool(name="sbuf", bufs=1) as pool,
            tc.tile_pool(name="psum", bufs=1, space="PSUM") as psum,
        ):
            Bt = pool.tile([128, _B_COLS], f32, name="Bt", tag="bt")
            Ht = pool.tile([128, 9], f32, name="Ht", tag="ht")
            St = pool.tile([128, 24], f32, name="St", tag="st")

            nc.sync.dma_start(Bt[:, :], b_dram[:, :])

            dims = [28, 28, 12, 3, 12, 12, 28, 40]
            ps = [psum.tile([m, 1], f32, name=f"ps{i}", tag=f"ps{i}")
                  for i, m in enumerate(dims)]

            def mm(i, lhs_col, k, m, rhs, start, stop):
                nc.tensor.matmul(ps[i][:, :], Bt[0:k, lhs_col:lhs_col + m],
                                 rhs, start=start, stop=stop)

            def tanh_to(i, m, hcol, func=AF.Tanh):
                nc.scalar.activation(Ht[0:m, hcol:hcol + 1], ps[i][0:m, 0:1],
                                     func, bias=0.0, scale=1.0)

            one = Bt[0:1, _C_ONE:_C_ONE + 1]
            EXTC = 7   # Ht col: ext vector [psi, x47, x52, 1.0]
            OUTC = 8   # Ht col: final output

            # --- atan2 DVE stage (ready right after the DMA). q is emitted
            # LAST so arctan's single DVE wait covers the whole chain. ---
            nc.vector.reciprocal(St[0:1, 0:2], Bt[0:1, _C_DEN:_C_DEN + 2])
            nc.vector.tensor_scalar(St[0:1, 4:6], Bt[0:1, _C_DEN:_C_DEN + 2],
                                    0.0, None, OP.is_lt)            # [den<0]
            nc.vector.tensor_scalar(St[0:1, 6:8], Bt[0:1, _C_NUM:_C_NUM + 2],
                                    0.0, None, OP.is_ge)            # [num>=0]
            nc.vector.tensor_scalar(St[0:1, 8:10], St[0:1, 6:8],
                                    2.0, -1.0, OP.mult, OP.add)     # sign(num)
            nc.vector.tensor_mul(St[0:1, 10:12], St[0:1, 8:10], St[0:1, 4:6])
            nc.vector.tensor_scalar(St[0:1, 12:14], St[0:1, 10:12],
                                    float(np.pi / 2), None, OP.mult)
            nc.vector.tensor_add(St[0:1, 14:15], St[0:1, 12:13],
                                 St[0:1, 13:14])                    # b = corr/2
            nc.vector.tensor_mul(St[0:1, 2:4], Bt[0:1, _C_NUM:_C_NUM + 2],
                                 St[0:1, 0:2])                      # q = num/den

            # --- L1 (bias in lhsT1 row 0 against v0[0]=1.0) ---
            mm(0, _C_L1, 84, 28, Bt[0:84, _C_V0:_C_V0 + 1], True, True)
            tanh_to(0, 28, 0)
            # ext tail [x47, x52, 1.0] -> Ht; also gives ACT its one DMA wait
            nc.scalar.activation(Ht[0:4, EXTC:EXTC + 1],
                                 Bt[0:4, _C_EXT:_C_EXT + 1], AF.Copy,
                                 bias=0.0, scale=1.0)
            # --- atan2 ACT tail, filling the tanh1->tanh2 gap:
            # psi = sum_f(arctan(q_f) + b) via Identity + accum_out.
            # b is bounced through an ACT copy so the psi op's waits stay
            # single-proc (walrus allows one sync wait per instruction). ---
            nc.scalar.activation(St[0:1, 20:21], St[0:1, 14:15], AF.Copy,
                                 bias=0.0, scale=1.0)
            nc.scalar.activation(St[0:1, 16:18], St[0:1, 2:4], AF.Arctan,
                                 bias=0.0, scale=1.0)
            nc.scalar.activation(St[0:1, 18:20], St[0:1, 16:18], AF.Identity,
                                 bias=St[0:1, 20:21], scale=1.0,
                                 accum_out=Ht[0:1, EXTC:EXTC + 1])

            # --- L2 ---
            mm(1, _C_B2, 1, 28, one, True, False)
            mm(1, _C_L2, 28, 28, Ht[0:28, 0:1], False, True)
            tanh_to(1, 28, 1)

            # --- L3 ---
            mm(2, _C_B3, 1, 12, one, True, False)
            mm(2, _C_L3, 28, 12, Ht[0:28, 1:2], False, True)
            tanh_to(2, 12, 2)
            # --- L4 ---
            mm(3, _C_B4, 1, 3, one, True, False)
            mm(3, _C_L4, 12, 3, Ht[0:12, 2:3], False, True)
            tanh_to(3, 3, 3)
            # --- L5: M5 (h4 + ext) + b5 ---
            mm(4, _C_L5B, 4, 12, Ht[0:4, EXTC:EXTC + 1], True, False)
            mm(4, _C_L5A, 3, 12, Ht[0:3, 3:4], False, True)
            tanh_to(4, 12, 4)
            # --- L6 ---
            mm(5, _C_B6, 1, 12, one, True, False)
            mm(5, _C_L6B, 12, 12, Ht[0:12, 2:3], False, False)
            mm(5, _C_L6A, 12, 12, Ht[0:12, 4:5], False, True)
            tanh_to(5, 12, 5)
            # --- L7 ---
            mm(6, _C_B7, 1, 28, one, True, False)
            mm(6, _C_L7B, 28, 28, Ht[0:28, 1:2], False, False)
            mm(6, _C_L7A, 12, 28, Ht[0:12, 5:6], False, True)
            tanh_to(6, 28, 6)
            # --- L8 (bias in lhsT8b row 0; no tanh) ---
            mm(7, _C_L8B, 84, 40, Bt[0:84, _C_V0:_C_V0 + 1], True, False)
            mm(7, _C_L8A, 28, 40, Ht[0:28, 6:7], False, True)
            tanh_to(7, 40, OUTC, func=AF.Copy)

            nc.sync.dma_start(out_dram[:, :], Ht[0:40, OUTC:OUTC + 1])

    nc.compile()
    _CACHE["nc"] = nc
    return nc


def _build_blk(inputs):
    blk = _build_w_block(inputs)
    _fill_x_block(blk, inputs["x"])
    return blk


def kernel(**inputs) -> np.ndarray:
    nc = _build_bass()
    blk = _build_blk(inputs)

    from concourse.bass_utils import run_bass_kernel_spmd

    res = run_bass_kernel_spmd(nc, [{"blk": blk.copy()} for _ in range(8)],
                               core_ids=list(range(8)))
    out = np.asarray(res.results[0]["out"], F32).reshape(1, 40)
    return out
